# revision 1
# baseline (speedup 1.0000x reference)
"""Head-parallel HGNN attention-coefficient kernel for Trainium2 (Bass/Tile).

Per head h (8 heads):
    Q = emb_dest @ Wq[h] + bq[h]            [4096, 512]
    K = emb_src  @ Wk[h] + bk[h]            [4096, 512]
    V = feat_src @ Wv[h] + bv[h]            [4096, 512]
    S = Q @ K^T / sqrt(512)                 [4096, 4096]
    O = elu(softmax(S, -1) @ V)             [4096, 512]
output = mean_h O                           [4096, 512]

Sharding: one head per NeuronCore (8 heads, 8 cores, zero redundant
compute, no collectives). The host transposes emb/feat once (shared by
all cores), casts matmul operands to bf16, and slices per-head weights;
the device computes Q^T/K^T (hidden dim on partitions) so the score
matmul produces S^T tiles (N_src on partitions) whose exp() feeds the
P@V matmul directly as the stationary operand — no on-device transpose
of the 4096x4096 attention matrix.  Softmax runs without max
subtraction (|S|max ~ 2.4 for this problem's distribution, exp is safe)
and the row normalizer is recovered with a ones-vector partition-sum
matmul; normalization and ELU are applied to the [4096, 512] output
tiles.  The host averages the 8 per-head outputs.
"""

import numpy as np
import ml_dtypes

P = 128
D = 512            # IN_DIM
E = 512            # HIDDEN
N = 4096           # N_DST
M = 4096           # N_SRC
H = 8
DC = D // P        # 4 contraction chunks for projections
EC = E // P        # 4
MC = M // P        # 32 N_src chunks
NSTRIP = 512       # N_dst columns handled per strip
NSTRIPS = N // NSTRIP
NCH = NSTRIP // P  # 4 N_dst chunks per strip
SCALE = 1.0 / float(np.sqrt(E))

_cache = {}


def _build_nc():
    import concourse.mybir as mybir
    import concourse.tile as tile
    from concourse import bacc

    f32 = mybir.dt.float32
    bf16 = mybir.dt.bfloat16
    AF = mybir.ActivationFunctionType
    ALU = mybir.AluOpType

    nc = bacc.Bacc(
        "TRN2",
        target_bir_lowering=False,
        debug=False,
        enable_asserts=False,
        num_devices=H,
    )

    embT_d_h = nc.dram_tensor("embT_dest", [D, N], bf16, kind="ExternalInput")
    embT_s_h = nc.dram_tensor("embT_src", [D, M], bf16, kind="ExternalInput")
    featT_h = nc.dram_tensor("featT_src", [E, M], bf16, kind="ExternalInput")
    wq_h = nc.dram_tensor("Wq", [D, E], bf16, kind="ExternalInput")
    wk_h = nc.dram_tensor("Wk", [D, E], bf16, kind="ExternalInput")
    wv_h = nc.dram_tensor("Wv", [E, E], bf16, kind="ExternalInput")
    bq_h = nc.dram_tensor("bq", [E], f32, kind="ExternalInput")
    bk_h = nc.dram_tensor("bk", [E], f32, kind="ExternalInput")
    bv_h = nc.dram_tensor("bv", [E], bf16, kind="ExternalInput")
    out_h = nc.dram_tensor("out", [N, E], f32, kind="ExternalOutput")

    embT_d = embT_d_h.ap().rearrange("(c p) n -> p c n", p=P)
    embT_s = embT_s_h.ap().rearrange("(c p) n -> p c n", p=P)
    featT = featT_h.ap().rearrange("(c p) n -> p c n", p=P)
    out_ap = out_h.ap()

    with tile.TileContext(nc) as tc:
        with (
            tc.tile_pool(name="wpool", bufs=1) as wpool,
            tc.tile_pool(name="cpool", bufs=1) as cpool,
            tc.tile_pool(name="big", bufs=1) as big_pool,
            tc.tile_pool(name="embx", bufs=4) as embx_pool,
            tc.tile_pool(name="pt", bufs=36) as pt_pool,
            tc.tile_pool(name="ep", bufs=3) as ep_pool,
            tc.tile_pool(name="sm", bufs=2) as sm_pool,
            tc.tile_pool(name="psA", bufs=3, space="PSUM") as psA,
            tc.tile_pool(name="psO", bufs=2, space="PSUM") as psO,
            tc.tile_pool(name="psSm", bufs=1, space="PSUM") as psSm,
            tc.tile_pool(name="psRt", bufs=1, space="PSUM") as psRt,
        ):
            # --- constants / weights ---
            wq_sb = wpool.tile([P, DC, E], bf16, name="wq_sb")
            nc.sync.dma_start(wq_sb[:], wq_h.ap().rearrange("(c p) e -> p c e", p=P))
            wk_sb = wpool.tile([P, DC, E], bf16, name="wk_sb")
            nc.sync.dma_start(wk_sb[:], wk_h.ap().rearrange("(c p) e -> p c e", p=P))
            wv_sb = wpool.tile([P, EC, E], bf16, name="wv_sb")
            nc.sync.dma_start(wv_sb[:], wv_h.ap().rearrange("(c p) e -> p c e", p=P))
            bq_sb = cpool.tile([P, EC], f32, name="bq_sb")
            nc.sync.dma_start(bq_sb[:], bq_h.ap().rearrange("(c p) -> p c", p=P))
            bk_sb = cpool.tile([P, EC], f32, name="bk_sb")
            nc.sync.dma_start(bk_sb[:], bk_h.ap().rearrange("(c p) -> p c", p=P))
            bv_sb = cpool.tile([1, E], bf16, name="bv_sb")
            nc.sync.dma_start(bv_sb[:], bv_h.ap().rearrange("(o e) -> o e", o=1))

            ones_row = cpool.tile([1, P], bf16, name="ones_row")
            nc.any.memset(ones_row[:], 1.0)
            ones_col = cpool.tile([P, 1], f32, name="ones_col")
            nc.any.memset(ones_col[:], 1.0)
            one_one = cpool.tile([1, 1], f32, name="one_one")
            nc.any.memset(one_one[:], 1.0)

            # --- persistent activations ---
            qt_sb = big_pool.tile([P, EC, N], bf16, name="qt_sb")  # Q^T
            kt_sb = big_pool.tile([P, EC, M], bf16, name="kt_sb")  # K^T
            v_sb = big_pool.tile([P, MC, E], bf16, name="v_sb")    # V

            # --- projections: Q^T = Wq^T @ embT_dest, K^T likewise ---
            for src_ap, w_sb, b_sb, dst in (
                (embT_d, wq_sb, bq_sb, qt_sb),
                (embT_s, wk_sb, bk_sb, kt_sb),
            ):
                for nt in range(N // NSTRIP):
                    et = embx_pool.tile([P, DC, NSTRIP], bf16, tag="embx", name="et")
                    nc.sync.dma_start(
                        et[:], src_ap[:, :, nt * NSTRIP : (nt + 1) * NSTRIP]
                    )
                    for ec in range(EC):
                        ps = psA.tile([P, NSTRIP], f32, tag="psA", name="ps")
                        for dc in range(DC):
                            nc.tensor.matmul(
                                ps[:],
                                lhsT=w_sb[:, dc, ec * P : (ec + 1) * P],
                                rhs=et[:, dc, :],
                                start=(dc == 0),
                                stop=(dc == DC - 1),
                            )
                        nc.scalar.activation(
                            dst[:, ec, nt * NSTRIP : (nt + 1) * NSTRIP],
                            ps[:],
                            AF.Identity,
                            bias=b_sb[:, ec : ec + 1],
                        )

            # --- projection: V = feat_src @ Wv + bv (bias via K=1 matmul) ---
            for mt in range(M // NSTRIP):
                ft = embx_pool.tile([P, EC, NSTRIP], bf16, tag="embx", name="ft")
                nc.sync.dma_start(ft[:], featT[:, :, mt * NSTRIP : (mt + 1) * NSTRIP])
                for mi in range(NSTRIP // P):
                    mc = mt * (NSTRIP // P) + mi
                    ps = psA.tile([P, E], f32, tag="psA", name="psv")
                    for ec in range(EC):
                        nc.tensor.matmul(
                            ps[:],
                            lhsT=ft[:, ec, mi * P : (mi + 1) * P],
                            rhs=wv_sb[:, ec, :],
                            start=(ec == 0),
                            stop=False,
                        )
                    nc.tensor.matmul(
                        ps[:], lhsT=ones_row[:], rhs=bv_sb[:], start=False, stop=True
                    )
                    nc.scalar.activation(v_sb[:, mc, :], ps[:], AF.Copy)

            # --- attention, one strip of 512 N_dst columns at a time ---
            for st in range(NSTRIPS):
                n0 = st * NSTRIP
                acc = sm_pool.tile([P, NSTRIP], f32, tag="acc", name="acc")
                pts = []
                for mc in range(MC):
                    ps = psA.tile([P, NSTRIP], f32, tag="psA", name="pss")
                    for ec in range(EC):
                        nc.tensor.matmul(
                            ps[:],
                            lhsT=kt_sb[:, ec, mc * P : (mc + 1) * P],
                            rhs=qt_sb[:, ec, n0 : n0 + NSTRIP],
                            start=(ec == 0),
                            stop=(ec == EC - 1),
                        )
                    ptt = pt_pool.tile([P, NSTRIP], bf16, tag="pt", name="ptt")
                    nc.scalar.activation(ptt[:], ps[:], AF.Exp, scale=SCALE)
                    pts.append(ptt)
                    # running partition-parallel sum of exp for the softmax denom
                    if mc == 0:
                        nc.vector.tensor_copy(acc[:], ptt[:])
                    else:
                        nc.vector.tensor_add(acc[:], acc[:], ptt[:])

                # denominators: column-sum over partitions, then move n onto
                # partitions via K=1 matmuls, then reciprocal
                cs_ps = psSm.tile([1, NSTRIP], f32, tag="cs", name="cs_ps")
                nc.tensor.matmul(
                    cs_ps[:], lhsT=ones_col[:], rhs=acc[:], start=True, stop=True
                )
                cs_sb = sm_pool.tile([1, NSTRIP], f32, tag="cs_sb", name="cs_sb")
                nc.vector.tensor_copy(cs_sb[:], cs_ps[:])
                rt_ps = psRt.tile([P, NCH], f32, tag="rt", name="rt_ps")
                for ncn in range(NCH):
                    nc.tensor.matmul(
                        rt_ps[:, ncn : ncn + 1],
                        lhsT=cs_sb[:, ncn * P : (ncn + 1) * P],
                        rhs=one_one[:],
                        start=True,
                        stop=True,
                    )
                rinv = sm_pool.tile([P, NCH], f32, tag="rinv", name="rinv")
                nc.vector.reciprocal(rinv[:], rt_ps[:])

                # O tile = sum_m exp(S^T)[m, n-chunk]^T @ V[m, :]
                for ncn in range(NCH):
                    po = psO.tile([P, E], f32, tag="psO", name="po")
                    for mc in range(MC):
                        nc.tensor.matmul(
                            po[:],
                            lhsT=pts[mc][:, ncn * P : (ncn + 1) * P],
                            rhs=v_sb[:, mc, :],
                            start=(mc == 0),
                            stop=(mc == MC - 1),
                        )
                    # normalize + ELU:  elu(x) = max(x,0) + min(exp(x),1) - 1
                    t0 = ep_pool.tile([P, E], f32, tag="t0", name="t0")
                    nc.vector.tensor_scalar_mul(t0[:], po[:], rinv[:, ncn : ncn + 1])
                    ex = ep_pool.tile([P, E], f32, tag="ex", name="ex")
                    nc.scalar.activation(ex[:], t0[:], AF.Exp)
                    nc.vector.tensor_scalar_max(t0[:], t0[:], 0.0)
                    nc.vector.tensor_scalar(ex[:], ex[:], 1.0, -1.0, ALU.min, ALU.add)
                    nc.vector.tensor_add(t0[:], t0[:], ex[:])
                    nc.sync.dma_start(out_ap[n0 + ncn * P : n0 + (ncn + 1) * P, :], t0[:])

    nc.compile()
    return nc


def _get_nc():
    nc = _cache.get("nc")
    if nc is None:
        nc = _build_nc()
        _cache["nc"] = nc
    return nc


def _make_in_maps(inputs):
    bf = ml_dtypes.bfloat16
    f32 = np.float32
    embT_d = np.asarray(inputs["emb_dest"], f32).T.astype(bf)
    embT_s = np.asarray(inputs["emb_src"], f32).T.astype(bf)
    featT = np.asarray(inputs["feat_src"], f32).T.astype(bf)
    Wq = np.asarray(inputs["Wq"], f32)
    Wk = np.asarray(inputs["Wk"], f32)
    Wv = np.asarray(inputs["Wv"], f32)
    bq = np.asarray(inputs["bq"], f32)
    bk = np.asarray(inputs["bk"], f32)
    bv = np.asarray(inputs["bv"], f32)
    in_maps = []
    for h in range(H):
        in_maps.append(
            {
                "embT_dest": embT_d,
                "embT_src": embT_s,
                "featT_src": featT,
                "Wq": Wq[h].astype(bf),
                "Wk": Wk[h].astype(bf),
                "Wv": Wv[h].astype(bf),
                "bq": np.ascontiguousarray(bq[h]),
                "bk": np.ascontiguousarray(bk[h]),
                "bv": bv[h].astype(bf),
            }
        )
    return in_maps


def kernel(**inputs):
    from concourse.bass_utils import run_bass_kernel_spmd

    nc = _get_nc()
    in_maps = _make_in_maps(inputs)
    res = run_bass_kernel_spmd(nc, in_maps, core_ids=list(range(H)))
    outs = np.stack([r["out"] for r in res.results], axis=0)
    return outs.mean(axis=0, dtype=np.float64).astype(np.float32)


# revision 7
# speedup vs baseline: 12.9075x; 12.9075x over previous
"""Head-parallel HGNN attention-coefficient kernel for Trainium2 (Bass/Tile).

Per head h (8 heads):
    Q = emb_dest @ Wq[h] + bq[h]            [4096, 512]
    K = emb_src  @ Wk[h] + bk[h]            [4096, 512]
    V = feat_src @ Wv[h] + bv[h]            [4096, 512]
    S = Q @ K^T / sqrt(512)                 [4096, 4096]
    O = elu(softmax(S, -1) @ V)             [4096, 512]
output = mean_h O                           [4096, 512]

Sharding: one head per NeuronCore (8 heads, 8 cores, zero redundant
compute, no collectives). The host transposes emb/feat once (shared by
all cores), casts matmul operands to bf16, and slices per-head weights;
the device computes Q^T/K^T (hidden dim on partitions) so the score
matmul produces S^T tiles (N_src on partitions) whose exp() feeds the
P@V matmul directly as the stationary operand — no on-device transpose
of the 4096x4096 attention matrix.  Softmax runs without max
subtraction (|S|max ~ 2.4 for this problem's distribution, exp is safe)
and the row normalizer is recovered with a ones-vector partition-sum
matmul; normalization and ELU are applied to the [4096, 512] output
tiles.  The host averages the 8 per-head outputs.
"""

import numpy as np
import ml_dtypes

P = 128
D = 512            # IN_DIM
E = 512            # HIDDEN
N = 4096           # N_DST
M = 4096           # N_SRC
H = 8
DC = D // P        # 4 contraction chunks for projections
EC = E // P        # 4
MC = M // P        # 32 N_src chunks
NSTRIP = 512       # N_dst columns handled per strip
NSTRIPS = N // NSTRIP
NCH = NSTRIP // P  # 4 N_dst chunks per strip
SCALE = 1.0 / float(np.sqrt(E))

_cache = {}


def _build_nc(repeat=1):
    import concourse.mybir as mybir
    import concourse.tile as tile
    from concourse import bacc

    f32 = mybir.dt.float32
    bf16 = mybir.dt.bfloat16
    AF = mybir.ActivationFunctionType
    ALU = mybir.AluOpType

    nc = bacc.Bacc(
        "TRN2",
        target_bir_lowering=False,
        debug=False,
        enable_asserts=False,
        num_devices=H,
    )

    embT_d_h = nc.dram_tensor("embT_dest", [D, N], bf16, kind="ExternalInput")
    embT_s_h = nc.dram_tensor("embT_src", [D, M], bf16, kind="ExternalInput")
    featT_h = nc.dram_tensor("featT_src", [E, M], bf16, kind="ExternalInput")
    wq_h = nc.dram_tensor("Wq", [D, E], bf16, kind="ExternalInput")
    wk_h = nc.dram_tensor("Wk", [D, E], bf16, kind="ExternalInput")
    wv_h = nc.dram_tensor("Wv", [E, E], bf16, kind="ExternalInput")
    bq_h = nc.dram_tensor("bq", [E], f32, kind="ExternalInput")
    bk_h = nc.dram_tensor("bk", [E], f32, kind="ExternalInput")
    bv_h = nc.dram_tensor("bv", [E], bf16, kind="ExternalInput")
    out_h = nc.dram_tensor("out", [N, E], f32, kind="ExternalOutput")

    embT_d = embT_d_h.ap().rearrange("(c p) n -> p c n", p=P)
    embT_s = embT_s_h.ap().rearrange("(c p) n -> p c n", p=P)
    featT = featT_h.ap().rearrange("(c p) n -> p c n", p=P)
    out_ap = out_h.ap()

    with tile.TileContext(nc) as tc:
        with (
            tc.tile_pool(name="wpool", bufs=1) as wpool,
            tc.tile_pool(name="cpool", bufs=1) as cpool,
            tc.tile_pool(name="big", bufs=1) as big_pool,
            tc.tile_pool(name="embx", bufs=6) as embx_pool,
            tc.tile_pool(name="pt", bufs=40) as pt_pool,
            tc.tile_pool(name="ep", bufs=4) as ep_pool,
            tc.tile_pool(name="sm", bufs=2) as sm_pool,
            tc.tile_pool(name="psA", bufs=4, space="PSUM") as psA,
            tc.tile_pool(name="psO", bufs=2, space="PSUM") as psO,
            tc.tile_pool(name="psSm", bufs=1, space="PSUM") as psSm,
            tc.tile_pool(name="psRt", bufs=1, space="PSUM") as psRt,
        ):
            # --- constants / weights ---
            wq_sb = wpool.tile([P, DC, E], bf16, name="wq_sb")
            nc.sync.dma_start(wq_sb[:], wq_h.ap().rearrange("(c p) e -> p c e", p=P))
            wk_sb = wpool.tile([P, DC, E], bf16, name="wk_sb")
            nc.sync.dma_start(wk_sb[:], wk_h.ap().rearrange("(c p) e -> p c e", p=P))
            wv_sb = wpool.tile([P, EC, E], bf16, name="wv_sb")
            nc.sync.dma_start(wv_sb[:], wv_h.ap().rearrange("(c p) e -> p c e", p=P))
            bq_sb = cpool.tile([P, EC], f32, name="bq_sb")
            nc.sync.dma_start(bq_sb[:], bq_h.ap().rearrange("(c p) -> p c", p=P))
            bk_sb = cpool.tile([P, EC], f32, name="bk_sb")
            nc.sync.dma_start(bk_sb[:], bk_h.ap().rearrange("(c p) -> p c", p=P))
            bv_sb = cpool.tile([1, E], bf16, name="bv_sb")
            nc.sync.dma_start(bv_sb[:], bv_h.ap().rearrange("(o e) -> o e", o=1))

            ones_row = cpool.tile([1, P], bf16, name="ones_row")
            nc.any.memset(ones_row[:], 1.0)
            ones_col = cpool.tile([P, 1], f32, name="ones_col")
            nc.any.memset(ones_col[:], 1.0)
            one_one = cpool.tile([1, 1], f32, name="one_one")
            nc.any.memset(one_one[:], 1.0)

            # (repeat > 1 re-runs the whole computation; used only by the
            # test harness to measure per-iteration HW time differentially)
            for _rep in range(repeat):
                # --- persistent activations ---
                qt_sb = big_pool.tile([P, EC, N], bf16, tag="qt", name="qt_sb")
                kt_sb = big_pool.tile([P, EC, M], bf16, tag="kt", name="kt_sb")
                v_sb = big_pool.tile([P, MC, E], bf16, tag="v", name="v_sb")

                # --- projections: Q^T = Wq^T @ embT_dest, K^T likewise ---
                for src_ap, w_sb, b_sb, dst in (
                    (embT_d, wq_sb, bq_sb, qt_sb),
                    (embT_s, wk_sb, bk_sb, kt_sb),
                ):
                    for nt in range(N // NSTRIP):
                        et = embx_pool.tile(
                            [P, DC, NSTRIP], bf16, tag="embx", name="et"
                        )
                        nc.sync.dma_start(
                            et[:], src_ap[:, :, nt * NSTRIP : (nt + 1) * NSTRIP]
                        )
                        for ec in range(EC):
                            ps = psA.tile([P, NSTRIP], f32, tag="psA", name="ps")
                            for dc in range(DC):
                                nc.tensor.matmul(
                                    ps[:],
                                    lhsT=w_sb[:, dc, ec * P : (ec + 1) * P],
                                    rhs=et[:, dc, :],
                                    start=(dc == 0),
                                    stop=(dc == DC - 1),
                                )
                            nc.scalar.activation(
                                dst[:, ec, nt * NSTRIP : (nt + 1) * NSTRIP],
                                ps[:],
                                AF.Identity,
                                bias=b_sb[:, ec : ec + 1],
                            )

                # --- projection: V = feat_src @ Wv + bv (bias as K=1 matmul) ---
                for mt in range(M // NSTRIP):
                    ft = embx_pool.tile([P, EC, NSTRIP], bf16, tag="embx", name="ft")
                    nc.sync.dma_start(
                        ft[:], featT[:, :, mt * NSTRIP : (mt + 1) * NSTRIP]
                    )
                    for mi in range(NSTRIP // P):
                        mc = mt * (NSTRIP // P) + mi
                        ps = psA.tile([P, E], f32, tag="psA", name="psv")
                        for ec in range(EC):
                            nc.tensor.matmul(
                                ps[:],
                                lhsT=ft[:, ec, mi * P : (mi + 1) * P],
                                rhs=wv_sb[:, ec, :],
                                start=(ec == 0),
                                stop=False,
                            )
                        nc.tensor.matmul(
                            ps[:],
                            lhsT=ones_row[:],
                            rhs=bv_sb[:],
                            start=False,
                            stop=True,
                        )
                        nc.scalar.activation(v_sb[:, mc, :], ps[:], AF.Copy)

                # --- attention, one strip of 512 N_dst columns at a time ---
                for st in range(NSTRIPS):
                    n0 = st * NSTRIP
                    acc = sm_pool.tile([P, NSTRIP], f32, tag="acc", name="acc")
                    pts = []
                    for mc in range(MC):
                        ps = psA.tile([P, NSTRIP], f32, tag="psA", name="pss")
                        for ec in range(EC):
                            nc.tensor.matmul(
                                ps[:],
                                lhsT=kt_sb[:, ec, mc * P : (mc + 1) * P],
                                rhs=qt_sb[:, ec, n0 : n0 + NSTRIP],
                                start=(ec == 0),
                                stop=(ec == EC - 1),
                            )
                        ptt = pt_pool.tile([P, NSTRIP], bf16, tag="pt", name="ptt")
                        nc.scalar.activation(ptt[:], ps[:], AF.Exp, scale=SCALE)
                        pts.append(ptt)
                        # running partition-parallel sum of exp (softmax denom)
                        if mc == 0:
                            nc.vector.tensor_copy(acc[:], ptt[:])
                        else:
                            nc.vector.tensor_add(acc[:], acc[:], ptt[:])

                    # denominators: column-sum over partitions, move n onto
                    # partitions via K=1 matmuls, then reciprocal
                    cs_ps = psSm.tile([1, NSTRIP], f32, tag="cs", name="cs_ps")
                    nc.tensor.matmul(
                        cs_ps[:], lhsT=ones_col[:], rhs=acc[:], start=True, stop=True
                    )
                    cs_sb = sm_pool.tile([1, NSTRIP], f32, tag="cs_sb", name="cs_sb")
                    nc.vector.tensor_copy(cs_sb[:], cs_ps[:])
                    rt_ps = psRt.tile([P, NCH], f32, tag="rt", name="rt_ps")
                    for ncn in range(NCH):
                        nc.tensor.matmul(
                            rt_ps[:, ncn : ncn + 1],
                            lhsT=cs_sb[:, ncn * P : (ncn + 1) * P],
                            rhs=one_one[:],
                            start=True,
                            stop=True,
                        )
                    rinv = sm_pool.tile([P, NCH], f32, tag="rinv", name="rinv")
                    nc.vector.reciprocal(rinv[:], rt_ps[:])

                    # O tile = sum_m exp(S^T)[m, n-chunk]^T @ V[m, :]
                    for ncn in range(NCH):
                        po = psO.tile([P, E], f32, tag="psO", name="po")
                        for mc in range(MC):
                            nc.tensor.matmul(
                                po[:],
                                lhsT=pts[mc][:, ncn * P : (ncn + 1) * P],
                                rhs=v_sb[:, mc, :],
                                start=(mc == 0),
                                stop=(mc == MC - 1),
                            )
                        # normalize + ELU: elu(x) = max(x,0) + min(exp(x),1) - 1
                        t0 = ep_pool.tile([P, E], f32, tag="t0", name="t0")
                        nc.vector.tensor_scalar_mul(
                            t0[:], po[:], rinv[:, ncn : ncn + 1]
                        )
                        ex = ep_pool.tile([P, E], f32, tag="ex", name="ex")
                        nc.scalar.activation(ex[:], t0[:], AF.Exp)
                        nc.vector.tensor_scalar_max(t0[:], t0[:], 0.0)
                        nc.vector.tensor_scalar(
                            ex[:], ex[:], 1.0, -1.0, ALU.min, ALU.add
                        )
                        nc.vector.tensor_add(t0[:], t0[:], ex[:])
                        nc.sync.dma_start(
                            out_ap[n0 + ncn * P : n0 + (ncn + 1) * P, :], t0[:]
                        )

    nc.compile()
    return nc


def _get_nc():
    nc = _cache.get("nc")
    if nc is None:
        nc = _build_nc()
        _cache["nc"] = nc
    return nc


def _make_in_maps(inputs):
    bf = ml_dtypes.bfloat16
    f32 = np.float32
    embT_d = np.asarray(inputs["emb_dest"], f32).T.astype(bf)
    embT_s = np.asarray(inputs["emb_src"], f32).T.astype(bf)
    featT = np.asarray(inputs["feat_src"], f32).T.astype(bf)
    Wq = np.asarray(inputs["Wq"], f32)
    Wk = np.asarray(inputs["Wk"], f32)
    Wv = np.asarray(inputs["Wv"], f32)
    bq = np.asarray(inputs["bq"], f32)
    bk = np.asarray(inputs["bk"], f32)
    bv = np.asarray(inputs["bv"], f32)
    in_maps = []
    for h in range(H):
        in_maps.append(
            {
                "embT_dest": embT_d,
                "embT_src": embT_s,
                "featT_src": featT,
                "Wq": Wq[h].astype(bf),
                "Wk": Wk[h].astype(bf),
                "Wv": Wv[h].astype(bf),
                "bq": np.ascontiguousarray(bq[h]),
                "bk": np.ascontiguousarray(bk[h]),
                "bv": bv[h].astype(bf),
            }
        )
    return in_maps


def kernel(**inputs):
    from concourse.bass_utils import run_bass_kernel_spmd

    nc = _get_nc()
    in_maps = _make_in_maps(inputs)
    res = run_bass_kernel_spmd(nc, in_maps, core_ids=list(range(H)))
    outs = np.stack([r["out"] for r in res.results], axis=0)
    return outs.mean(axis=0, dtype=np.float64).astype(np.float32)
